# revision 35
# baseline (speedup 1.0000x reference)
"""Trainium2 Bass kernel for nn_App_Enc (attention pooling + weighted recombine).

Reference computation (per sample b):
    p        = softmax(raw_tps, axis=-1)                      # [N, S, S], per-row over w
    app_vec  = einsum('khw,nhw->nk', x, p)                    # [N, K]
    num      = einsum('nhw,nk->khw', fitted_cj, app_vec)      # [K, S, S]
    denom    = 1 + sum_n fitted_cj                            # [S, S]
    out      = num / denom                                    # [K, S, S]

Sharding: pure data parallel over batch B=16 -> 8 cores x 2 samples.
"""

import numpy as np

import concourse.bass as bass
import concourse.tile as tile
from concourse import mybir
from concourse.bass_utils import run_bass_kernel_spmd

# Problem constants (hardcoded per harness contract).
B = 16
N_HM = 32
K_APP = 16
S_FULL = 256
N_CORES = 8
BPC = B // N_CORES  # samples per core = 2

P = 128  # SBUF partitions
F32 = mybir.dt.float32
F32R = mybir.dt.float32r
BF16 = mybir.dt.bfloat16


def build_program(S=S_FULL, bpc=BPC, parts="full"):
    """Build the per-core Bass program. All 8 cores run the same graph on
    their own 2-sample shard; there is no cross-core communication."""
    HW = S * S
    WB = S // P           # w-blocks per row (2 for S=256)
    assert S % P == 0
    NROWS = N_HM * S      # (n,h) rows per sample
    RSUB = 4              # 128-row sub-tiles per raw super-tile
    NSUP = NROWS // (P * RSUB)
    CPS = HW // P         # hw chunks of 128 per sample (contraction tiles)
    XU = 8                # x sub-tiles per x super-tile
    XG = 8                # chunks packed per x sub-tile
    XSUP = CPS // (XU * XG)
    F2 = 512              # einsum-2 moving free dim (one PSUM bank of f32)
    CH2 = HW // F2        # einsum-2 hw chunks (each covers BOTH samples)
    GC = 8                # chunks per packed output group (4096 hw)
    NG = CH2 // GC        # output windows of 4096 hw
    NGH = NG // 2         # einsum-2 processes chunk-pairs (c, c + CH2/2)
    M2 = 2 * bpc * K_APP  # einsum-2 out rows: (chunk-half q, sample s, k) blocks

    nc = bass.Bass()

    x_ext = nc.declare_dram_parameter("x", [bpc, K_APP, S, S], F32, isOutput=False)
    raw_ext = nc.declare_dram_parameter("raw", [bpc, N_HM, S, S], F32, isOutput=False)
    fit_ext = nc.declare_dram_parameter("fit", [bpc, N_HM, S, S], F32, isOutput=False)
    idn_ext = nc.declare_dram_parameter("idn", [P, P], F32, isOutput=False)
    p4_ext = nc.declare_dram_parameter("p4", [P, M2], F32, isOutput=False)
    out_ext = nc.declare_dram_parameter("out", [bpc, K_APP, S, S], F32, isOutput=True)

    x_flat = [x_ext[b].rearrange("k h w -> k (h w)") for b in range(bpc)]
    raw_flat = [raw_ext[b].rearrange("n h w -> (n h) w") for b in range(bpc)]
    fit_flat = fit_ext[:].rearrange("s n h w -> s n (h w)")
    out_flat = out_ext[:].rearrange("s k h w -> s k (h w)")

    with tile.TileContext(nc) as tc:
        import contextlib

        ctx = contextlib.ExitStack()
        with ctx:
            singles = ctx.enter_context(tc.tile_pool(name="singles", bufs=1))
            xt_pool = ctx.enter_context(tc.tile_pool(name="xt", bufs=2))
            pt_pool = ctx.enter_context(tc.tile_pool(name="pt", bufs=3))
            av_pool = ctx.enter_context(tc.tile_pool(name="avs", bufs=2))
            raw_pool = ctx.enter_context(tc.tile_pool(name="raw", bufs=3))
            pe_pool = ctx.enter_context(tc.tile_pool(name="p_exp", bufs=3))
            ps_pool = ctx.enter_context(tc.tile_pool(name="p_scaled", bufs=3))
            sums_pool = ctx.enter_context(tc.tile_pool(name="sums", bufs=3))
            xs_pool = ctx.enter_context(tc.tile_pool(name="xs", bufs=3))

            # phase 1-4 PSUM pools: closed before phase 5 so its pools can
            # use the freed banks
            ctx14 = contextlib.ExitStack()
            tp_psum = ctx14.enter_context(
                tc.tile_pool(name="tp_psum", bufs=2, space="PSUM")
            )
            av_psum = ctx14.enter_context(
                tc.tile_pool(name="av_psum", bufs=2, space="PSUM")
            )

            # --- identity (fp32 + bf16 copies), av_aug stationary ---
            idn_f32 = singles.tile([P, P], F32)
            nc.sync.dma_start(out=idn_f32[:], in_=idn_ext[:])
            idn_bf = singles.tile([P, P], BF16)
            nc.vector.tensor_copy(out=idn_bf[:], in_=idn_f32[:])

            zbias = singles.tile([P, 1], F32)
            nc.vector.memset(zbias[:], 0)
            # block-diagonal stationary for einsum-2: rows (q, s, n), cols (q, s, k)
            av_aug = singles.tile([2 * bpc * N_HM, M2], BF16)
            nc.vector.memset(av_aug[:], 0)
            # ones-block [128, 4]: OB4[32g + n, g'] = (g == g'), so
            # dp4 = OB4^T @ ft gives the 4 (q, s)-group denominators compactly
            oblk4 = singles.tile([2 * bpc * N_HM, 2 * bpc], BF16)
            nc.vector.memset(oblk4[:], 0)
            for b in range(2 * bpc):
                nc.vector.memset(
                    oblk4[b * N_HM : (b + 1) * N_HM, b : b + 1], 1.0
                )
            # expansion pattern, repeated at every 32-row offset so lhsT and
            # rhs share a base partition: P4[32c + i, 16g + k] = (g == i).
            # Host-provided (engine memsets cannot address unaligned
            # partition bases); converted to bf16 to match the rt operand.
            p4_f32 = singles.tile([P, M2], F32)
            nc.sync.dma_start(out=p4_f32[:], in_=p4_ext[:])
            p4 = singles.tile([P, M2], BF16)
            nc.vector.tensor_copy(out=p4[:], in_=p4_f32[:])
            # [1, *] ones rows for the PSUM ones-init matmul (the +1 of the
            # denominator): init = ones_col^T @ ones_row
            ones_row = singles.tile([1, F2], BF16)
            nc.vector.memset(ones_row[:], 1.0)
            ones_col = singles.tile([1, P], BF16)
            nc.vector.memset(ones_col[:], 1.0)

            xt_tiles = []
            for s in range(bpc):
                # ---------- phase 1: softmax -> p (bf16, normalized) ----------
                pt_tiles = [
                    pt_pool.tile([P, N_HM, S], BF16, tag="pt", name=f"pt_{s}_{wh}")
                    for wh in range(WB)
                ]
                for T in range(NSUP):
                    rtile = raw_pool.tile([P, RSUB, S], F32, tag="raw")
                    src = raw_flat[s][T * (P * RSUB) : (T + 1) * (P * RSUB), :]
                    nc.sync.dma_start(
                        out=rtile[:], in_=src.rearrange("(a p) w -> p a w", p=P)
                    )
                    petile = pe_pool.tile([P, RSUB, S], BF16, tag="pe")
                    rowsum = sums_pool.tile([P, RSUB], F32, tag="rowsum")
                    rz = sums_pool.tile([P, RSUB], F32, tag="rz")
                    for a in range(RSUB):
                        nc.scalar.activation(
                            out=petile[:, a, :],
                            in_=rtile[:, a, :],
                            func=mybir.ActivationFunctionType.Exp,
                            bias=zbias[:],
                            accum_out=rowsum[:, a : a + 1],
                        )
                    nc.vector.reciprocal(out=rz[:], in_=rowsum[:])
                    pstile = ps_pool.tile([P, RSUB, S], BF16, tag="ps")
                    for a in range(RSUB):
                        nc.vector.tensor_scalar_mul(
                            out=pstile[:, a, :],
                            in0=petile[:, a, :],
                            scalar1=rz[:, a : a + 1],
                        )
                    # ---------- phase 2: transpose p blocks ----------
                    # 128-row block `blk` holds rows of a single n: n = blk//(S/P),
                    # h-range (blk % (S/P))*P. In the [n, h]-flat free space of a
                    # pt tile, block blk lands at offset blk*P. 4 transposes per
                    # PSUM bank -> one 512-wide evacuation.
                    blk = RSUB * T
                    for wh in range(WB):
                        # transpose mode keeps PSUM in bf16, so the evac copy
                        # runs in the DVE 2x perf mode
                        tp = tp_psum.tile([P, RSUB, P], BF16, tag="tp")
                        for i in range(RSUB):
                            nc.tensor.transpose(
                                out=tp[:, i, :],
                                in_=pstile[:, i, wh * P : (wh + 1) * P],
                                identity=idn_bf[:],
                            )
                        nc.vector.tensor_copy(
                            out=pt_tiles[wh][:].rearrange("p n h -> p (n h)")[
                                :, blk * P : (blk + RSUB) * P
                            ],
                            in_=tp[:].rearrange("p a b -> p (a b)"),
                        )

                # ---------- phase 3: transpose x ----------
                # xs partitions are (k, g): k-major keeps the DMA source AP at
                # 3 dims ((u j) collapse into one contiguous run of 1024).
                # xt_all[p, T, g, u, k] holds x^T for chunk c = XU*XG*T + XG*g + u
                xt_all = xt_pool.tile([P, XSUP, XG, XU, K_APP], BF16, tag="xt")
                xt_tiles.append(xt_all)
                for T in range(XSUP):
                    xst = xs_pool.tile([P, XU, P], F32, tag="xs")
                    src = x_flat[s].rearrange(
                        "k (t g uj) -> t k g uj", t=XSUP, g=XG, uj=XU * P
                    )[T]
                    nc.sync.dma_start(
                        out=xst[:].rearrange("p u j -> p (u j)"),
                        in_=src,
                    )
                    # bf16 staging on Pool (walrus needs matching matmul
                    # dtypes), then bf16 transposes + DVE 2x-mode evac
                    xsb = xs_pool.tile([P, XU, P], BF16, tag="xsb")
                    nc.gpsimd.tensor_copy(out=xsb[:], in_=xst[:])
                    for u0 in range(0, XU, 4):
                        tp = tp_psum.tile([P, 4, P], BF16, tag="tp")
                        for i in range(4):
                            nc.tensor.transpose(
                                out=tp[:, i, :],
                                in_=xsb[:, u0 + i, :],
                                identity=idn_bf[:],
                            )
                        # psum cols iterate (i, k, g); scatter into xt_all
                        nc.vector.tensor_copy(
                            out=xt_all[:, T, :, u0 : u0 + 4, :].transpose(
                                [0, 2, 3, 1]
                            ),
                            in_=tp[:].rearrange("p a b -> p (a b)"),
                        )

                # ---------- phase 4: app_vec accumulation ----------
                if parts == "sm":
                    continue
                avp = av_psum.tile([K_APP, N_HM], F32, tag="av")
                n_mm = 0
                for wh in range(WB):
                    for h in range(S):
                        c = WB * h + wh
                        nc.tensor.matmul(
                            out=avp[:],
                            lhsT=xt_all[:, c // (XU * XG), (c % (XU * XG)) // XU, c % XU, :],
                            rhs=pt_tiles[wh][:, :, h],
                            start=(n_mm == 0),
                            stop=(n_mm == CPS - 1),
                        )
                        n_mm += 1
                # evacuate app_vec^T, transpose to [N, K], write into av_aug blocks
                avt_sb = av_pool.tile([K_APP, N_HM], F32, tag="avt")
                nc.vector.tensor_copy(out=avt_sb[:], in_=avp[:])
                av2 = av_psum.tile([N_HM, K_APP], F32, tag="av")
                nc.tensor.matmul(
                    out=av2[:],
                    lhsT=avt_sb[:],
                    rhs=idn_f32[0:K_APP, 0:K_APP],
                    start=True,
                    stop=True,
                )
                for q in range(2):
                    nc.vector.tensor_copy(
                        out=av_aug[(q * bpc + s) * N_HM : (q * bpc + s + 1) * N_HM,
                                   (q * bpc + s) * K_APP : (q * bpc + s + 1) * K_APP],
                        in_=av2[:],
                    )

            # free phase 1-4 PSUM banks for the phase-5 pipeline
            ctx14.close()

            fit_pool = ctx.enter_context(tc.tile_pool(name="fit", bufs=7))
            r_pool = ctx.enter_context(tc.tile_pool(name="rpool", bufs=4))
            rxe_pool = ctx.enter_context(tc.tile_pool(name="rxe", bufs=4))
            sg_pool = ctx.enter_context(tc.tile_pool(name="sg", bufs=2))
            d_psum = ctx.enter_context(
                tc.tile_pool(name="d_psum", bufs=2, space="PSUM")
            )
            rx_psum = ctx.enter_context(
                tc.tile_pool(name="rx_psum", bufs=3, space="PSUM")
            )
            e2_psum = ctx.enter_context(
                tc.tile_pool(name="e2_psum", bufs=3, space="PSUM")
            )

            # ---------- phase 5: einsum-2 + denom + divide ----------
            if parts in ("sm", "sm_av"):
                ot0 = singles.tile([1, 16], F32)
                nc.vector.memset(ot0[:], 0)
                nc.sync.dma_start(out=out_flat[0, :, 0:1].transpose([1, 0]), in_=ot0[:])
                parts_skip_e2 = True
            else:
                parts_skip_e2 = False
            # Each matmul contracts 128 = (q, s, n) rows: two hw-chunks (c and
            # c + CH2/2) of both samples at once, via the block-diag stationary.
            av_r = av_aug[:]
            fit_src = fit_flat.rearrange(
                "s n (q gg f) -> gg q s n f", q=2, f=GC * F2
            )
            HFC = GC // 2  # chunks per fitted half-tile
            for gh in range((2 * NGH) if not parts_skip_e2 else 0):
                g, hf = gh // 2, gh % 2
                ft = fit_pool.tile([2 * bpc * N_HM, HFC * F2], BF16, tag="fit")
                for q in range(2):
                    # Act/SWDGE issue stream: its own queue, so prefetches
                    # don't head-of-line block behind out-DMAs (sync) or the
                    # divide muls (Pool)
                    nc.gpsimd.dma_start(
                        out=ft[q * bpc * N_HM : (q + 1) * bpc * N_HM, :],
                        in_=fit_src[g, q][:, :, hf * (HFC * F2) : (hf + 1) * (HFC * F2)],
                    )
                # --- denominators, packed 2 chunks per PSUM bank (PE tile
                # offsets allow only 0/32/64): bank pre-set to 1.0 (the +1)
                # by a rank-1 ones matmul, then chunk 2*bh+i accumulates its
                # 4 (q, s)-group sums at partition offset 32*i. One
                # reciprocal serves two chunks.
                rts = []
                for bh in range(HFC // 2):
                    dall = d_psum.tile([M2, F2], F32, tag="dall")
                    nc.tensor.matmul(
                        out=dall[:], lhsT=ones_col[:, :M2], rhs=ones_row[:],
                        start=True, stop=False, skip_group_check=True,
                    )
                    for i in range(2):
                        cl = 2 * bh + i
                        nc.tensor.matmul(
                            out=dall[i * 32 : i * 32 + 2 * bpc, :],
                            lhsT=oblk4[:],
                            rhs=ft[:, cl * F2 : (cl + 1) * F2],
                            start=False, stop=True, skip_group_check=True,
                        )
                    rt = r_pool.tile([M2, F2], BF16, tag="rt")
                    with nc.allow_low_precision("bf16 reciprocal of ~17-magnitude denominator; output tolerance 2e-2"):
                        nc.vector.reciprocal(out=rt[:], in_=dall[:])
                    rts.append(rt)
                stg = sg_pool.tile([M2, HFC * F2], F32, tag="stg")
                for cl in range(HFC):
                    # f32r moving operand: full PE rate at free dim >= 256
                    ftc = ft[:, cl * F2 : (cl + 1) * F2]
                    # replicate this chunk's 4 reciprocal rows to the
                    # (q, s, k)-row layout
                    i = cl % 2
                    rx = rx_psum.tile([M2, F2], F32, tag="rx")
                    nc.tensor.matmul(
                        out=rx[:],
                        lhsT=p4[i * 32 : i * 32 + 2 * bpc, :],
                        rhs=rts[cl // 2][i * 32 : i * 32 + 2 * bpc, :],
                        start=True, stop=True,
                    )
                    # TensorTensor may read only one PSUM input: stage the
                    # expanded reciprocal in SBUF via Act (idle in the tail)
                    rxe = rxe_pool.tile([M2, F2], BF16, tag="rxe")
                    nc.scalar.copy(out=rxe[:], in_=rx[:])
                    # numerator straight from fitted (divide happens at evac)
                    ep = e2_psum.tile([M2, F2], F32, tag="e2")
                    nc.tensor.matmul(
                        out=ep[:], lhsT=av_r, rhs=ftc,
                        start=True, stop=True,
                    )
                    # fused divide + PSUM evacuation (DVE: Pool cannot
                    # read PSUM)
                    nc.vector.tensor_mul(
                        out=stg[:, cl * F2 : (cl + 1) * F2],
                        in0=ep[:], in1=rxe[:],
                    )
                for q in range(2):
                    w_g = g + q * NGH  # 4096-wide output hw window
                    off = w_g * (GC * F2) + hf * (HFC * F2)
                    nc.sync.dma_start(
                        out=out_flat[:, :, off : off + HFC * F2],
                        in_=stg[q * bpc * K_APP : (q + 1) * bpc * K_APP, :],
                    )

    return nc


# Walrus in this toolchain accepts at most ONE sync-wait on datapath
# instructions; hoist excess waits onto standalone sequencer EventSemaphore
# instructions (the same thing raw-bass wait_ge emits).
_SEQ_OPS = {"EventSemaphore", "Branch", "SemaphoreOp", "Call",
            "EventSemaphoreRangeClear", "PseudoSyncBarrier", "Halt", "Notify"}


def _legalize_sync_waits(d, max_waits=1):
    for fn in d["functions"]:
        for blk in fn["blocks"]:
            out = []
            for ins in blk["instructions"]:
                si = ins.get("sync_info")
                w = (si or {}).get("on_wait") or []
                if si and len(w) > max_waits and ins.get("opcode") not in _SEQ_OPS:
                    extra, keep = w[:-max_waits], w[-max_waits:]
                    for j, ew in enumerate(extra):
                        out.append({
                            "debug": ins.get("debug", 0),
                            "engine": ins["engine"],
                            "ins": [], "outs": [],
                            "name": f"{ins['name']}-esw{j}",
                            "opcode": "EventSemaphore",
                            "sync_info": {"on_update": [], "on_wait": [ew]},
                        })
                    si["on_wait"] = keep
                out.append(ins)
            blk["instructions"] = out
    return d


def _patch_serialization(nc):
    import json as _json

    orig = nc.to_json_bytes

    def patched():
        d = _json.loads(orig())
        _legalize_sync_waits(d)
        return _json.dumps(d).encode()

    nc.to_json_bytes = patched
    return nc


_CACHE = {}


def _get_program():
    key = (S_FULL, BPC)
    if key not in _CACHE:
        _CACHE[key] = _patch_serialization(build_program())
    return _CACHE[key]


_RUN_OPTS = {}  # test harness may set {"trace": True}
LAST_RESULT = None


def aux_inputs():
    idn = np.eye(P, dtype=np.float32)
    p4 = np.zeros((P, 2 * BPC * K_APP), dtype=np.float32)
    for c4 in range(P // 32):
        for i in range(2 * BPC):
            p4[c4 * 32 + i, i * K_APP : (i + 1) * K_APP] = 1.0
    return {"idn": idn, "p4": p4}


def kernel(x, raw_tps, fitted_cj):
    global LAST_RESULT
    nc = _get_program()
    aux = aux_inputs()
    in_maps = []
    for core in range(N_CORES):
        b0 = core * BPC
        in_maps.append(
            {
                "x": np.ascontiguousarray(x[b0 : b0 + BPC]),
                "raw": np.ascontiguousarray(raw_tps[b0 : b0 + BPC]),
                "fit": np.ascontiguousarray(fitted_cj[b0 : b0 + BPC]),
                **aux,
            }
        )
    res = run_bass_kernel_spmd(
        nc, in_maps, core_ids=list(range(N_CORES)), **_RUN_OPTS
    )
    LAST_RESULT = res
    outs = [
        np.asarray(res.results[i]["out"]).reshape(BPC, K_APP, S_FULL, S_FULL)
        for i in range(N_CORES)
    ]
    return np.concatenate(outs, axis=0)

